# revision 39
# baseline (speedup 1.0000x reference)
"""Trainium2 Bass kernel for nn_CombinedRepeatCausalLinear (bf16 version).

Math: out[r, t] = sum_{s<=t} x[r, s] * (w0[s]*dv0^(t-s) + w1[t]*dv1^(t-s)) + bias[t]

Chunked linear-attention formulation (chunk L=126 along S):
  - Diagonal blocks D_c[s_l, t_l] (upper-triangular, [128,128] with 2 extra
    "reduction" columns producing decay-weighted chunk sums S0_c, S1_c in
    rows 126/127 of the diagonal result).
  - Cross-chunk contribution is rank-2 per source chunk:
      out[t] += w1[t]*dv1^(t-e_c')*S0_c'[r] + dv0^(t-e_c')*S1_c'[r]
    implemented as a second matmul per chunk against a host-built matrix M
    whose contraction rows hold the per-chunk sums (gathered in `sall`).

Everything on the wire and in SBUF is bf16 (tolerance is 2e-2; bf16
end-to-end gives ~3e-3): halves HBM traffic vs fp32 and makes every
matmul single-pass (fp32 needs HI/LO 2-pass on the PE).  PSUM stays fp32.

Engine assignment (each FD=1024 chunk-op in one instruction):
  sync   : x chunk loads only (prefetch never blocked by output deps)
  scalar : D/M loads, PSUM->SBUF copies (ACT activation-copy), output stores
  vector : cross+diag merge adds (TENSOR_TENSOR, PSUM operand)
  gpsimd : chunk-sum extraction SBUF->SBUF DMAs, sall bias memset
  tensor : 2 diag + 2 cross matmuls per chunk (N=512, K<=128, bf16)

Phase-2 for chunk c-2 is emitted inside chunk c's loop iteration so the
vector/scalar/DMA work overlaps the input stream instead of trailing it.

Data-parallel across 8 NeuronCores on the fused B*E axis.
"""

import sys

if "/opt/trn_rl_repo" not in sys.path:
    sys.path.insert(0, "/opt/trn_rl_repo")

import ml_dtypes
import numpy as np

import concourse.mybir as mybir
from concourse import bacc
from concourse.bass_utils import run_bass_kernel_spmd
from concourse.mybir import AluOpType
from concourse.tile import TileContext

_P = 128
_B, _E, _S = 4, 2048, 2048
_NCORES = 8
_R = (_B * _E) // _NCORES  # 1024 rows (r) per core
_L = 126  # chunk length along S
_NCH = (_S + _L - 1) // _L  # 17 chunks (last has 32)
_HALF = 512  # r per matmul (one PSUM bank, fp32)
_XROWS = _NCH * _P  # 2176 padded x rows (>= 126*16+128)
_OROWS = _NCH * _P  # padded out rows (>= 126*17)

_F32 = mybir.dt.float32
_BF16 = mybir.dt.bfloat16
_npbf = ml_dtypes.bfloat16


def _chunk_len(c):
    return min(_L, _S - c * _L)


def _build_host_mats(w0, w1, dv0, dv1, bias, with_bias):
    """Build D [128, NCH*128] (partition-major, SBUF layout: one line-rate DMA)
    and M [srows, NCH*128] in float64, cast bf16."""
    w0 = w0.astype(np.float64)
    w1 = w1.astype(np.float64)
    D = np.zeros((_NCH * _P, _P), dtype=np.float64)
    for c in range(_NCH):
        Lc = _chunk_len(c)
        base = c * _L
        sl = np.arange(Lc)
        tl = np.arange(Lc)
        diff = tl[None, :] - sl[:, None]
        mask = diff >= 0
        blk = np.where(
            mask,
            w0[base + sl][:, None] * (dv0 ** np.maximum(diff, 0))
            + w1[base + tl][None, :] * (dv1 ** np.maximum(diff, 0)),
            0.0,
        )
        Db = D[c * _P : (c + 1) * _P]
        Db[:Lc, :Lc] = blk
        # reduction columns: col 126 -> S0_c (dv1-weighted sum),
        #                    col 127 -> S1_c (w0*dv0-weighted sum)
        Db[:Lc, 126] = dv1 ** (Lc - 1 - sl)
        Db[:Lc, 127] = w0[base + sl] * dv0 ** (Lc - 1 - sl)

    off = 1 if with_bias else 0
    srows = off + 2 * _NCH
    M = np.zeros((srows, _NCH * _P), dtype=np.float64)
    for c in range(_NCH):
        Lc = _chunk_len(c)
        t = c * _L + np.arange(Lc)
        if with_bias:
            M[0, c * _P : c * _P + Lc] = bias.astype(np.float64)[t]
        for cp in range(c):
            e_cp = cp * _L + _chunk_len(cp) - 1
            M[off + 2 * cp, c * _P : c * _P + Lc] = w1[t] * (dv1 ** (t - e_cp))
            M[off + 2 * cp + 1, c * _P : c * _P + Lc] = dv0 ** (t - e_cp)
    # partition-major D: D_sb[p, c*128 + t] = D[c*128 + p, t]
    D_sb = np.ascontiguousarray(
        D.reshape(_NCH, _P, _P).transpose(1, 0, 2).reshape(_P, _NCH * _P)
    )
    # M replicated at partition offset 64 so the two per-chunk cross matmuls
    # (K <= 35) can run concurrently in disjoint PE row-strips
    M2 = np.zeros((64 + srows, _NCH * _P), dtype=np.float64)
    M2[:srows] = M
    M2[64 : 64 + srows] = M
    return D_sb.astype(_npbf), M2.astype(_npbf)


def _build(with_bias):
    off = 1 if with_bias else 0
    srows = off + 2 * _NCH
    nc = bacc.Bacc(
        "TRN2",
        target_bir_lowering=False,
        debug=False,
        enable_asserts=False,
        num_devices=_NCORES,
    )
    xt = nc.dram_tensor("xt", [_XROWS, _R], _BF16, kind="ExternalInput").ap()
    Dd = nc.dram_tensor("Dd", [_P, _NCH * _P], _BF16, kind="ExternalInput").ap()
    Md = nc.dram_tensor(
        "Md", [64 + srows, _NCH * _P], _BF16, kind="ExternalInput"
    ).ap()
    outT = nc.dram_tensor("outT", [_OROWS, _R], _BF16, kind="ExternalOutput").ap()

    with TileContext(nc) as tc:
        with (
            tc.tile_pool(name="consts", bufs=1) as cpool,
            tc.tile_pool(name="xin", bufs=12) as xpool,
            tc.tile_pool(name="pd", bufs=4, space="PSUM") as pdpool,
            tc.tile_pool(name="po", bufs=2, space="PSUM") as popool,
        ):
            # D + M on the scalar queue (idle until the first PSUM copy, and
            # HWDGE-fast) so they stream concurrently with x on sync; the
            # first two D blocks load separately so diag(0)/diag(1) aren't
            # gated on the full 544KB transfer
            Dt = cpool.tile([_P, _NCH * _P], _BF16)
            nc.scalar.dma_start(Dt[:, 0 : 2 * _P], Dd[:, 0 : 2 * _P])
            nc.scalar.dma_start(Dt[:, 2 * _P :], Dd[:, 2 * _P :])
            Mt = cpool.tile([64 + srows, _NCH * _P], _BF16)
            sall = cpool.tile([64 + srows, _R], _BF16)
            if with_bias:
                nc.vector.memset(sall[0:1, :], 1.0)
                nc.vector.memset(sall[64:65, :], 1.0)

            # all chunk outputs (bf16) live in one big tile; sub-views keep
            # hazards per-region and let output DMAs cover 2 chunks at once
            dgall = cpool.tile([_P, _NCH * _R], _BF16)

            def dgv(c):
                return dgall[:, c * _R : (c + 1) * _R]

            def store_pair(j, eng):
                # output store for chunks {j-1, j}; emitted 2 steps after
                # phase2(j) so the queue never stalls extracts on it
                eng.dma_start(
                    outT[(j - 1) * _L : (j + 1) * _L, :].rearrange(
                        "(c p) r -> p c r", c=2
                    ),
                    dgall[0:_L, (j - 1) * _R : (j + 1) * _R].rearrange(
                        "p (c r) -> p c r", c=2
                    ),
                )

            def phase2(j):
                dg = dgv(j)
                kj = off + 2 * j
                if kj > 0:
                    po = popool.tile([_P, _R], _F32, tag="po", name="po")
                    # two cross matmuls at disjoint PE row-strips (base
                    # partitions 0 and 64) run concurrently on the array
                    for h, base in ((0, 0), (1, 64)):
                        nc.tensor.matmul(
                            po[:, h * _HALF : (h + 1) * _HALF],
                            Mt[base : base + kj, j * _P : (j + 1) * _P],
                            sall[base : base + kj, h * _HALF : (h + 1) * _HALF],
                            start=True,
                            stop=True,
                        )
                    nc.vector.tensor_tensor(
                        dg[0:_L, :], dg[0:_L, :], po[0:_L, :], AluOpType.add
                    )

            for c in range(_NCH):
                xtile = xpool.tile([_P, _R], _BF16, tag="x", name="x")
                nc.sync.dma_start(xtile[:], xt[c * _L : c * _L + _P, :])
                if c == 0:
                    nc.scalar.dma_start(Mt[:], Md[:])
                dg = dgv(c)
                for h in range(2):
                    pd = pdpool.tile([_P, _HALF], _F32, tag="pd", name="pd")
                    nc.tensor.matmul(
                        pd[:],
                        Dt[:, c * _P : (c + 1) * _P],
                        xtile[:, h * _HALF : (h + 1) * _HALF],
                        start=True,
                        stop=True,
                    )
                    nc.scalar.copy(dg[:, h * _HALF : (h + 1) * _HALF], pd[:])
                if c < _NCH - 1:
                    nc.sync.dma_start(
                        sall[off + 2 * c : off + 2 * c + 2, :], dg[126:128, :]
                    )
                    nc.gpsimd.dma_start(
                        sall[64 + off + 2 * c : 64 + off + 2 * c + 2, :],
                        dg[126:128, :],
                    )
                if c >= 1:
                    phase2(c - 1)
                if c >= 4 and (c - 3) % 2 == 1:
                    store_pair(c - 3, nc.gpsimd)
            # tail stores go on sync so gpsimd's last DMA retires early and
            # its expensive DGE drain overlaps the remaining tail work
            phase2(_NCH - 1)
            store_pair(_NCH - 2, nc.sync)  # chunks {14, 15}
            nc.sync.dma_start(
                outT[(_NCH - 1) * _L : _NCH * _L, :], dgv(_NCH - 1)[0:_L, :]
            )
    nc.compile()
    return nc


def _run(x, weight, bias, decay_value, trace=False):
    x = np.asarray(x, dtype=np.float32)
    w = np.asarray(weight, dtype=np.float32)
    b = np.asarray(bias, dtype=np.float32)
    dv = np.asarray(decay_value, dtype=np.float32)
    dv0 = float(np.clip(dv[0, 0], 0.9, 1.0))
    dv1 = float(np.clip(dv[1, 0], 0.9, 1.0))
    with_bias = bool(np.any(b))

    D, M = _build_host_mats(w[0], w[1], dv0, dv1, b, with_bias)
    nc = _build(with_bias)

    xf = x.reshape(_B * _E, _S)
    xTb = np.zeros((_XROWS, _B * _E), dtype=_npbf)
    xTb[:_S] = xf.T.astype(_npbf)
    in_maps = []
    for c in range(_NCORES):
        in_maps.append(
            {
                "xt": np.ascontiguousarray(xTb[:, c * _R : (c + 1) * _R]),
                "Dd": D,
                "Md": M,
            }
        )

    res = run_bass_kernel_spmd(nc, in_maps, core_ids=list(range(_NCORES)), trace=trace)
    outT = np.concatenate(
        [np.asarray(res.results[c]["outT"]) for c in range(_NCORES)], axis=1
    )  # [_OROWS, B*E] bf16
    full = np.ascontiguousarray(outT[:_S].T, dtype=np.float32).reshape(_B, _E, _S)
    return full, res


def kernel(x, weight, bias, decay_value):
    full, _ = _run(x, weight, bias, decay_value, trace=False)
    return full


# revision 40
# speedup vs baseline: 1.0033x; 1.0033x over previous
"""Trainium2 Bass kernel for nn_CombinedRepeatCausalLinear (bf16 version).

Math: out[r, t] = sum_{s<=t} x[r, s] * (w0[s]*dv0^(t-s) + w1[t]*dv1^(t-s)) + bias[t]

Chunked linear-attention formulation (chunk L=126 along S):
  - Diagonal blocks D_c[s_l, t_l] (upper-triangular, [128,128] with 2 extra
    "reduction" columns producing decay-weighted chunk sums S0_c, S1_c in
    rows 126/127 of the diagonal result).
  - Cross-chunk contribution is rank-2 per source chunk:
      out[t] += w1[t]*dv1^(t-e_c')*S0_c'[r] + dv0^(t-e_c')*S1_c'[r]
    implemented as a second matmul per chunk against a host-built matrix M
    whose contraction rows hold the per-chunk sums (gathered in `sall`).

Everything on the wire and in SBUF is bf16 (tolerance is 2e-2; bf16
end-to-end gives ~3e-3): halves HBM traffic vs fp32 and makes every
matmul single-pass (fp32 needs HI/LO 2-pass on the PE).  PSUM stays fp32.

The two cross matmuls of a chunk (K = off+2j <= 35) are row-packed into
disjoint PE array strips (base partitions 0 and 64) so they run
concurrently in one N=512 time slot; this needs M and the chunk-sum
table `sall` replicated at partition offset 64.

Engine assignment:
  sync   : x chunk loads, primary chunk-sum extracts, tail output stores
  scalar : D (split so diag(0) isn't gated on all of it) + M loads,
           PSUM->SBUF half-copies (ACT activation-copy)
  vector : cross+diag merge adds (TENSOR_TENSOR, PSUM operand)
  gpsimd : replica chunk-sum extracts, paired output stores (its
           expensive SWDGE drain overlaps the tail because its last DMA
           retires early)
  tensor : 2 diag matmuls + 1 packed cross slot per chunk

Phase-2 for chunk c-1 is emitted inside chunk c's loop iteration so the
vector/DMA work overlaps the input stream instead of trailing it.

Data-parallel across 8 NeuronCores on the fused B*E axis.
"""

import sys

if "/opt/trn_rl_repo" not in sys.path:
    sys.path.insert(0, "/opt/trn_rl_repo")

import ml_dtypes
import numpy as np

import concourse.mybir as mybir
from concourse import bacc
from concourse.bass_utils import run_bass_kernel_spmd
from concourse.mybir import AluOpType
from concourse.tile import TileContext

_P = 128
_B, _E, _S = 4, 2048, 2048
_NCORES = 8
_R = (_B * _E) // _NCORES  # 1024 rows (r) per core
_L = 126  # chunk length along S
_NCH = (_S + _L - 1) // _L  # 17 chunks (last has 32)
_HALF = 512  # r per matmul (one PSUM bank, fp32)
_XROWS = _NCH * _P  # 2176 padded x rows (>= 126*16+128)
_OROWS = _NCH * _P  # padded out rows (>= 126*17)

_F32 = mybir.dt.float32
_BF16 = mybir.dt.bfloat16
_npbf = ml_dtypes.bfloat16


def _chunk_len(c):
    return min(_L, _S - c * _L)


def _build_host_mats(w0, w1, dv0, dv1, bias, with_bias):
    """Build D [128, NCH*128] (partition-major, SBUF layout: one line-rate DMA)
    and M [srows, NCH*128] in float64, cast bf16."""
    w0 = w0.astype(np.float64)
    w1 = w1.astype(np.float64)
    D = np.zeros((_NCH * _P, _P), dtype=np.float64)
    for c in range(_NCH):
        Lc = _chunk_len(c)
        base = c * _L
        sl = np.arange(Lc)
        tl = np.arange(Lc)
        diff = tl[None, :] - sl[:, None]
        mask = diff >= 0
        blk = np.where(
            mask,
            w0[base + sl][:, None] * (dv0 ** np.maximum(diff, 0))
            + w1[base + tl][None, :] * (dv1 ** np.maximum(diff, 0)),
            0.0,
        )
        Db = D[c * _P : (c + 1) * _P]
        Db[:Lc, :Lc] = blk
        # reduction columns: col 126 -> S0_c (dv1-weighted sum),
        #                    col 127 -> S1_c (w0*dv0-weighted sum)
        Db[:Lc, 126] = dv1 ** (Lc - 1 - sl)
        Db[:Lc, 127] = w0[base + sl] * dv0 ** (Lc - 1 - sl)

    off = 1 if with_bias else 0
    srows = off + 2 * _NCH
    M = np.zeros((srows, _NCH * _P), dtype=np.float64)
    for c in range(_NCH):
        Lc = _chunk_len(c)
        t = c * _L + np.arange(Lc)
        if with_bias:
            M[0, c * _P : c * _P + Lc] = bias.astype(np.float64)[t]
        for cp in range(c):
            e_cp = cp * _L + _chunk_len(cp) - 1
            M[off + 2 * cp, c * _P : c * _P + Lc] = w1[t] * (dv1 ** (t - e_cp))
            M[off + 2 * cp + 1, c * _P : c * _P + Lc] = dv0 ** (t - e_cp)
    # partition-major D: D_sb[p, c*128 + t] = D[c*128 + p, t]
    D_sb = np.ascontiguousarray(
        D.reshape(_NCH, _P, _P).transpose(1, 0, 2).reshape(_P, _NCH * _P)
    )
    # M replicated at partition offset 64 so the two per-chunk cross matmuls
    # (K <= 35) can run concurrently in disjoint PE row-strips
    M2 = np.zeros((64 + srows, _NCH * _P), dtype=np.float64)
    M2[:srows] = M
    M2[64 : 64 + srows] = M
    return D_sb.astype(_npbf), M2.astype(_npbf)


def _build(with_bias):
    off = 1 if with_bias else 0
    srows = off + 2 * _NCH
    nc = bacc.Bacc(
        "TRN2",
        target_bir_lowering=False,
        debug=False,
        enable_asserts=False,
        num_devices=_NCORES,
    )
    xt = nc.dram_tensor("xt", [_XROWS, _R], _BF16, kind="ExternalInput").ap()
    Dd = nc.dram_tensor("Dd", [_P, _NCH * _P], _BF16, kind="ExternalInput").ap()
    Md = nc.dram_tensor(
        "Md", [64 + srows, _NCH * _P], _BF16, kind="ExternalInput"
    ).ap()
    outT = nc.dram_tensor("outT", [_OROWS, _R], _BF16, kind="ExternalOutput").ap()

    with TileContext(nc) as tc:
        with (
            tc.tile_pool(name="consts", bufs=1) as cpool,
            tc.tile_pool(name="xin", bufs=12) as xpool,
            tc.tile_pool(name="pd", bufs=4, space="PSUM") as pdpool,
            tc.tile_pool(name="po", bufs=2, space="PSUM") as popool,
        ):
            # D + M on the scalar queue (idle until the first PSUM copy, and
            # HWDGE-fast) so they stream concurrently with x on sync; the
            # first two D blocks load separately so diag(0)/diag(1) aren't
            # gated on the full 544KB transfer
            Dt = cpool.tile([_P, _NCH * _P], _BF16)
            nc.scalar.dma_start(Dt[:, 0 : 2 * _P], Dd[:, 0 : 2 * _P])
            nc.scalar.dma_start(Dt[:, 2 * _P :], Dd[:, 2 * _P :])
            Mt = cpool.tile([64 + srows, _NCH * _P], _BF16)
            sall = cpool.tile([64 + srows, _R], _BF16)
            if with_bias:
                nc.vector.memset(sall[0:1, :], 1.0)
                nc.vector.memset(sall[64:65, :], 1.0)

            # all chunk outputs (bf16) live in one big tile; sub-views keep
            # hazards per-region and let output DMAs cover 2 chunks at once
            dgall = cpool.tile([_P, _NCH * _R], _BF16)

            def dgv(c):
                return dgall[:, c * _R : (c + 1) * _R]

            def store_pair(j, eng):
                # output store for chunks {j-1, j}; emitted 2 steps after
                # phase2(j) so the queue never stalls extracts on it
                eng.dma_start(
                    outT[(j - 1) * _L : (j + 1) * _L, :].rearrange(
                        "(c p) r -> p c r", c=2
                    ),
                    dgall[0:_L, (j - 1) * _R : (j + 1) * _R].rearrange(
                        "p (c r) -> p c r", c=2
                    ),
                )

            def phase2(j):
                dg = dgv(j)
                kj = off + 2 * j
                if kj > 0:
                    po = popool.tile([_P, _R], _F32, tag="po", name="po")
                    # two cross matmuls at disjoint PE row-strips (base
                    # partitions 0 and 64) run concurrently on the array
                    for h, base in ((0, 0), (1, 64)):
                        nc.tensor.matmul(
                            po[:, h * _HALF : (h + 1) * _HALF],
                            Mt[base : base + kj, j * _P : (j + 1) * _P],
                            sall[base : base + kj, h * _HALF : (h + 1) * _HALF],
                            start=True,
                            stop=True,
                        )
                    nc.vector.tensor_tensor(
                        dg[0:_L, :], dg[0:_L, :], po[0:_L, :], AluOpType.add
                    )

            for c in range(_NCH):
                xtile = xpool.tile([_P, _R], _BF16, tag="x", name="x")
                nc.sync.dma_start(xtile[:], xt[c * _L : c * _L + _P, :])
                if c == 0:
                    nc.scalar.dma_start(Mt[:], Md[:])
                dg = dgv(c)
                for h in range(2):
                    pd = pdpool.tile([_P, _HALF], _F32, tag="pd", name="pd")
                    nc.tensor.matmul(
                        pd[:],
                        Dt[:, c * _P : (c + 1) * _P],
                        xtile[:, h * _HALF : (h + 1) * _HALF],
                        start=True,
                        stop=True,
                    )
                    nc.scalar.copy(dg[:, h * _HALF : (h + 1) * _HALF], pd[:])
                if c < _NCH - 1:
                    nc.sync.dma_start(
                        sall[off + 2 * c : off + 2 * c + 2, :], dg[126:128, :]
                    )
                    nc.gpsimd.dma_start(
                        sall[64 + off + 2 * c : 64 + off + 2 * c + 2, :],
                        dg[126:128, :],
                    )
                if c >= 1:
                    phase2(c - 1)
                if c >= 4 and (c - 3) % 2 == 1:
                    store_pair(c - 3, nc.gpsimd)
            # tail stores go on sync so gpsimd's last DMA retires early and
            # its expensive DGE drain overlaps the remaining tail work
            phase2(_NCH - 1)
            store_pair(_NCH - 2, nc.sync)  # chunks {14, 15}
            nc.sync.dma_start(
                outT[(_NCH - 1) * _L : _NCH * _L, :], dgv(_NCH - 1)[0:_L, :]
            )
    nc.compile()
    return nc


def _run(x, weight, bias, decay_value, trace=False):
    x = np.asarray(x, dtype=np.float32)
    w = np.asarray(weight, dtype=np.float32)
    b = np.asarray(bias, dtype=np.float32)
    dv = np.asarray(decay_value, dtype=np.float32)
    dv0 = float(np.clip(dv[0, 0], 0.9, 1.0))
    dv1 = float(np.clip(dv[1, 0], 0.9, 1.0))
    with_bias = bool(np.any(b))

    D, M = _build_host_mats(w[0], w[1], dv0, dv1, b, with_bias)
    nc = _build(with_bias)

    xf = x.reshape(_B * _E, _S)
    xTb = np.zeros((_XROWS, _B * _E), dtype=_npbf)
    xTb[:_S] = xf.T.astype(_npbf)
    in_maps = []
    for c in range(_NCORES):
        in_maps.append(
            {
                "xt": np.ascontiguousarray(xTb[:, c * _R : (c + 1) * _R]),
                "Dd": D,
                "Md": M,
            }
        )

    res = run_bass_kernel_spmd(nc, in_maps, core_ids=list(range(_NCORES)), trace=trace)
    outT = np.concatenate(
        [np.asarray(res.results[c]["outT"]) for c in range(_NCORES)], axis=1
    )  # [_OROWS, B*E] bf16
    full = np.ascontiguousarray(outT[:_S].T, dtype=np.float32).reshape(_B, _E, _S)
    return full, res


def kernel(x, weight, bias, decay_value):
    full, _ = _run(x, weight, bias, decay_value, trace=False)
    return full
